# revision 50
# baseline (speedup 1.0000x reference)
"""Trainium2 Bass kernel for nn_ConditionalSoftmax (sampled-softmax NLL loss).

Computes, for each batch row b:
    v_c   = vectors[cs[b]]                      # [D]
    h     = relu(v_c @ W1 + b1)                 # [H]
    logit = h @ W2 + b2                         # [V]
    nll_b = logsumexp(logit) - logit[v2s[ws[b]]]

Sharding: data-parallel over batch across 8 NeuronCores (1024 rows/core),
weights replicated.  Per core the dominant work is the [1024,512]@[512,20000]
matmul; it runs in fp8(e4m3) DoubleRow perf mode (2 fp8 weights per PE cell,
2 MACs/cycle), with W2 pre-scaled by 16 on the host so its values sit in the
e4m3 normal range.  W2 (10 MB in fp8) is preloaded once into SBUF and stays
resident; phase 2 runs batch-tile-outer with zero DMA.  Logits accumulate in
PSUM as [128,2048] slabs (4 banks); 7 of 10 slabs per batch tile are reduced
by the ScalarEngine's fused exp+row-sum (accum_out, scale=1/16 undoes the W2
scaling, written back in place over PSUM to avoid the ACT SBUF write bubble),
and 3 are reduced on the VectorEngine with a bias-tuned Schraudolph fast exp
(int bit-trick; only ~1e-4 relative error on each slab *sum*), so the two
engines together keep pace with the PE and the [1024,20000] logit matrix
never touches HBM.  The target logit takes a separate exact path:
indirect-gather of the needed W2.T rows (fp32) and a multiply-reduce on the
VectorEngine against an fp32 recompute of h, keeping NLL error ~1e-3.

All GPSIMD indirect-gather ops are emitted before the first activation in
program order: their DGE ring-management branches split the CFG, and any
split between activations makes the conservative act-table-load pass reload
the 1.3us exp table at every boundary.
"""

import numpy as np
import ml_dtypes

import concourse.bass as bass
import concourse.mybir as mybir
import concourse.tile as tile
from concourse import bacc, bass_utils
from concourse.bass import IndirectOffsetOnAxis, ts
from concourse.masks import make_identity

# Problem shapes (hardcoded per contest contract)
N_VOCAB = 50000
V = 20000
D = 300
DP = 384          # D padded to 3*128
NDC = 3           # contraction chunks for D
H = 512
NHC = 4           # contraction chunks for H (128 each)
NKP = 2           # DoubleRow contraction pairs (256 each)
B = 8192
NCORES = 8
BL = B // NCORES  # 1024 rows per core
NBT = BL // 128   # 8 batch tiles of 128 rows
W2SCALE = 16.0    # host-side scale on fp8 W2; undone by the Exp pre-scale

# Phase-2 vocab grouping: PSUM slabs of 2048 fp32 (4 banks), matmul chunks
# of <=512 so no matmul output crosses a PSUM bank. 20000 = 9*2048 + 1568.
VG = 2048
VGROUPS = [VG] * 9 + [20000 - 9 * VG]   # last = 1568
NVG = len(VGROUPS)
NW2DMA = 20       # W2 preload split for DMA-queue parallelism

F32 = mybir.dt.float32
BF16 = mybir.dt.bfloat16
FP8 = mybir.dt.float8e4
I32 = mybir.dt.int32
AF = mybir.ActivationFunctionType
OP = mybir.AluOpType
DR = mybir.MatmulPerfMode.DoubleRow


# Degree-4 least-squares fit of R(m) = log2(m) - m + 1 on [1,2], used by the
# DVE log path in phase 3 (computing ln on the ScalarEngine would force a
# ~1.3us act-table switch away from the exp table at every tile boundary).
_m_grid = np.linspace(1.0, 2.0, 100001)
_LOG_POLY = np.polyfit(_m_grid, np.log2(_m_grid) - _m_grid + 1.0, 4).tolist()
LN2 = float(np.log(2.0))


def _schraudolph_consts():
    """Constants for exp(x/16) ~= bitcast_f32(int(x*A + B)) on PSUM values
    x = 16*logit, used by the DVE fast-exp that reduces one slab per batch
    tile.  B's offset is tuned (fp64, through an fp32-faithful pipeline) so
    the *mean* multiplicative error over the logit distribution is ~0,
    making slab-sum errors ~1e-4 instead of the pointwise ~3%."""
    ln2 = float(np.log(2.0))
    l = np.linspace(-3.5, 3.5, 400001, dtype=np.float64)
    w = np.exp(-((l / 0.72) ** 2) / 2.0) * np.exp(l)
    A = (2.0 ** 23) / ln2
    C = 0.0
    for _ in range(10):
        v = np.float32(l * A + (127.0 * 2 ** 23 - C)).astype(np.float64)
        y = np.round(v).astype(np.int64).astype(np.uint32).view(np.float32)
        r = float(np.average(y.astype(np.float64) / np.exp(l), weights=w))
        if abs(r - 1.0) < 1e-9:
            break
        C += np.log2(r) * 2 ** 23
    return float(A / W2SCALE), float(127.0 * 2 ** 23 - C)


SCH_A, SCH_B = _schraudolph_consts()

_BUILD_CACHE = {}


def _chunks(width):
    """512-wide matmul chunks covering [0, width)."""
    out = []
    lo = 0
    while lo < width:
        w = min(512, width - lo)
        out.append((lo, w))
        lo += w
    return out


def _build(b1_nz: bool, b2_nz: bool):
    key = (b1_nz, b2_nz)
    if key in _BUILD_CACHE:
        return _BUILD_CACHE[key]

    nc = bacc.Bacc(
        "TRN2",
        target_bir_lowering=False,
        debug=False,
        num_devices=NCORES,
        num_swdge_queues=4,
    )

    cs_idx = nc.dram_tensor("cs_idx", [NBT, 128, 1], I32, kind="ExternalInput").ap()
    ws_idx = nc.dram_tensor("ws_idx", [NBT, 128, 1], I32, kind="ExternalInput").ap()
    vectors = nc.dram_tensor("vectors", [N_VOCAB, D], F32, kind="ExternalInput").ap()
    v2s = nc.dram_tensor("v2s", [N_VOCAB, 1], I32, kind="ExternalInput").ap()
    w1 = nc.dram_tensor("w1", [DP, H], BF16, kind="ExternalInput").ap()
    w2 = nc.dram_tensor("w2", [H, V], FP8, kind="ExternalInput").ap()
    w2tb = nc.dram_tensor("w2tb", [V, H + 1], F32, kind="ExternalInput").ap()
    if b1_nz:
        b1c = nc.dram_tensor("b1c", [NHC, 128, 1], F32, kind="ExternalInput").ap()
        b1rep = nc.dram_tensor("b1rep", [128, H], F32, kind="ExternalInput").ap()
    if b2_nz:
        b2rep = nc.dram_tensor("b2rep", [128, V], F32, kind="ExternalInput").ap()
    # partition-major [128, NBT]: one contiguous output DMA; host transposes
    nll = nc.dram_tensor("nll", [128, NBT], F32, kind="ExternalOutput").ap()

    with tile.TileContext(nc) as tc:
        with (
            tc.tile_pool(name="consts", bufs=1) as consts,
            tc.tile_pool(name="idx", bufs=8) as idxp,
            tc.tile_pool(name="vc", bufs=8) as vcp,
            tc.tile_pool(name="gw", bufs=4) as gwp,
            tc.tile_pool(name="scr", bufs=2) as scrp,
            # Single PSUM pool/tag: 2 bufs x [128,2048] fp32 = all 8 banks.
            # Phase-1 tiles draw smaller shapes from the same tag.
            tc.tile_pool(name="ps", bufs=2, space="PSUM") as psp,
        ):
            ident = consts.tile([128, 128], BF16)
            w1sb = consts.tile([128, NDC, H], BF16)
            if b1_nz:
                b1sb = consts.tile([128, NHC], F32)
                for hc in range(NHC):
                    nc.sync.dma_start(b1sb[:, hc : hc + 1], b1c[hc])
                b1rep_sb = consts.tile([128, H], F32)
                nc.sync.dma_start(b1rep_sb[:], b1rep[:])

            # Long-lived activations / resident weights.  W2 is resident in
            # SBUF as one tile per phase-2 slab group: a single big tile
            # would make every slab's first matmul wait on ALL preload DMAs
            # (tile-granular dependency), pinning the first exp to the end
            # of the whole 10 MB load.
            w2tiles = [
                consts.tile(
                    [128, NHC, VGROUPS[vg]], FP8,
                    name=f"w2sb{vg}", tag=f"w2_{vg}",
                )
                for vg in range(NVG)
            ]
            vcT = consts.tile([128, NDC, BL], BF16)    # v_c^T, d-major
            hT = consts.tile([128, NHC, BL], FP8)      # h^T, h-major (PE fp8 input)
            hb = consts.tile([128, NBT, H], F32)       # h, batch-major (target dot)
            sums = consts.tile([128, NBT * NVG], F32)  # per-(b,vg) exp partial sums
            tdot = consts.tile([128, NBT], F32)        # target logits
            fin = consts.tile([128, 7 * NBT], F32)     # DVE-log scratch columns
            if b2_nz:
                b2sb = consts.tile([128, V], F32)
                nc.sync.dma_start(b2sb[:], b2rep[:])

            # ---- Phase 1 front helpers.  All GPSIMD indirect gathers must be
            # emitted before the first activation (see module docstring), and
            # the half-0 compute must be emitted before the half-1 gathers:
            # the tile scheduler coalesces semaphore waits over everything
            # emitted earlier on the producer engine, so any gather emitted
            # before the first transpose would gate the whole PE pipeline. ----
            cidxs, vcs, vcbs, gs = [], [], [], [None] * NBT
            for t in range(NBT):
                cidx = idxp.tile([128, 1], I32, tag="cidx")
                nc.sync.dma_start(cidx[:], cs_idx[t])
                cidxs.append(cidx)
            # w1 load after the cidx DMAs: anything queued ahead of them
            # delays the first gather (waits are coalesced per queue).
            w1r = w1.rearrange("(c p) h -> p c h", p=128)
            for c in range(NDC):
                nc.sync.dma_start(w1sb[:, c, :], w1r[:, c, :])

            def gather_cast(t):
                vc = vcp.tile([128, D], F32, tag="vc")
                nc.gpsimd.indirect_dma_start(
                    out=vc[:],
                    out_offset=None,
                    in_=vectors[:],
                    in_offset=IndirectOffsetOnAxis(ap=cidxs[t][:, :1], axis=0),
                )
                vcb = vcp.tile([128, DP], BF16, tag="vcb")
                nc.vector.memset(vcb[:, D:DP], 0.0)
                nc.vector.tensor_copy(vcb[:, :D], vc[:])
                vcs.append(vc)
                vcbs.append(vcb)

            def gather_targets():
                for t in range(NBT):
                    widx = idxp.tile([128, 1], I32, tag="widx")
                    nc.sync.dma_start(widx[:], ws_idx[t])
                    sidx = idxp.tile([128, 1], I32, tag="sidx")
                    nc.gpsimd.indirect_dma_start(
                        out=sidx[:],
                        out_offset=None,
                        in_=v2s[:],
                        in_offset=IndirectOffsetOnAxis(ap=widx[:, :1], axis=0),
                    )
                    g = gwp.tile([128, H + 1], F32, tag="g", bufs=8)
                    nc.gpsimd.indirect_dma_start(
                        out=g[:],
                        out_offset=None,
                        in_=w2tb[:],
                        in_offset=IndirectOffsetOnAxis(ap=sidx[:, :1], axis=0),
                    )
                    gs[t] = g

            def phase1a_block(tb, n_t):
                # transposes + first layer for batch tiles tb..tb+n_t-1; all
                # transposes land in ONE psum tile (c-major) so the PSUM ring
                # doesn't ping-pong PE<->DVE per 128-col tile, then one wide
                # copy moves them to vcT.  h^T in fp8 slabs [128h x 128*n_t b];
                # relu+bias+cast on the DVE.  Called with n_t=1 for t=0 so
                # phase 2 starts after a single gather.
                w = 128 * n_t
                lo = 128 * tb
                pt3 = psp.tile([128, NDC, w], BF16, tag="ps")
                for c in range(NDC):
                    for j in range(n_t):
                        nc.tensor.transpose(
                            pt3[:, c, ts(j, 128)],
                            vcbs[tb + j][:, ts(c, 128)],
                            ident[:],
                        )
                nc.vector.tensor_copy(vcT[:, :, lo : lo + w], pt3[:])
                for hc in range(NHC):
                    ph = psp.tile([128, w], F32, tag="ps")
                    for c in range(NDC):
                        nc.tensor.matmul(
                            ph[:],
                            lhsT=w1sb[:, c, ts(hc, 128)],
                            rhs=vcT[:, c, lo : lo + w],
                            start=(c == 0),
                            stop=(c == NDC - 1),
                        )
                    if b1_nz:
                        nc.vector.tensor_scalar(
                            out=hT[:, hc, lo : lo + w],
                            in0=ph[:],
                            scalar1=b1sb[:, hc : hc + 1],
                            scalar2=0.0,
                            op0=OP.add,
                            op1=OP.max,
                        )
                    else:
                        nc.vector.tensor_scalar_max(
                            hT[:, hc, lo : lo + w], ph[:], 0.0
                        )

            def phase1_hb_block(tb, n_t):
                # batch-major h (fp32) for the target-logit dot; emitted
                # after later phase2_t calls so it doesn't delay the
                # first exp slabs (hb is only needed by the target dot).
                for t in range(tb, tb + n_t):
                    phb = psp.tile([128, H], F32, tag="ps")
                    for c in range(NDC):
                        nc.tensor.matmul(
                            phb[:],
                            lhsT=vcT[:, c, ts(t, 128)],
                            rhs=w1sb[:, c, :],
                            start=(c == 0),
                            stop=(c == NDC - 1),
                        )
                    if b1_nz:
                        nc.vector.tensor_add(phb[:], phb[:], b1rep_sb[:])
                    nc.vector.tensor_scalar_max(hb[:, t, :], phb[:], 0.0)

            def phase1b_tdot():
                # target-logit dot on the DVE; emitted after phase2_t(1) so
                # these ops don't sit in the DVE queue ahead of the copies /
                # relus that gate the PE pipeline during phase 1.
                for t in range(NBT):
                    # (tensor_tensor_reduce is broken on this HW; use 3 ops)
                    gscr = gwp.tile([128, H], F32, tag="gscr")
                    nc.vector.tensor_mul(gscr[:], hb[:, t, :], gs[t][:, :H])
                    gacc = gwp.tile([128, 1], F32, tag="gacc")
                    nc.vector.reduce_sum(
                        out=gacc[:], in_=gscr[:], axis=mybir.AxisListType.X
                    )
                    nc.vector.tensor_add(
                        tdot[:, t : t + 1], gacc[:], gs[t][:, H : H + 1]
                    )

            def phase2_slab(t, vg, vgw, v0, dve):
                ps = psp.tile([128, VG], F32, tag="ps")
                # kc-outer so the stationary hT slice is reused across
                # the 4 column chunks of the slab.
                for kc in range(NKP):
                    for lo, w in _chunks(vgw):
                        nc.tensor.matmul(
                            ps[:, lo : lo + w],
                            lhsT=hT[:, 2 * kc : 2 * kc + 2, ts(t, 128)],
                            rhs=w2tiles[vg][:, 2 * kc : 2 * kc + 2, lo : lo + w],
                            start=(kc == 0),
                            stop=(kc == NKP - 1),
                            perf_mode=DR,
                        )
                if b2_nz:
                    nc.vector.tensor_add(
                        ps[:, :vgw], ps[:, :vgw], b2sb[:, v0 : v0 + vgw]
                    )
                acc = sums[:, t * NVG + vg : t * NVG + vg + 1]
                if dve:
                    scr = scrp.tile([128, VG], I32, tag="scr", bufs=3)
                    nc.vector.tensor_scalar(
                        out=scr[:, :vgw],
                        in0=ps[:, :vgw],
                        scalar1=SCH_A,
                        scalar2=SCH_B,
                        op0=OP.mult,
                        op1=OP.add,
                    )
                    nc.vector.reduce_sum(
                        out=acc, in_=scr[:, :vgw].bitcast(F32),
                        axis=mybir.AxisListType.X,
                    )
                else:
                    nc.scalar.activation(
                        ps[:, :vgw], ps[:, :vgw], AF.Exp,
                        scale=1.0 / W2SCALE,
                        accum_out=acc,
                    )

            def phase2_pair(ta, tb):
                # first two batch tiles interleaved vg-outer: one tile alone
                # consumes the 10MB resident W2 (~19us) faster than the
                # preload DMAs deliver it (~28us), so a t-outer start would
                # leave the PE waiting on the load.  Two tiles per vocab
                # chunk halve the consumption rate to below DMA supply.
                v0 = 0
                for vg, vgw in enumerate(VGROUPS):
                    phase2_slab(ta, vg, vgw, v0, vg in (3, 7))
                    phase2_slab(tb, vg, vgw, v0, vg in (1, 5, 9))
                    v0 += vgw

            def phase2_t(t):
                # DoubleRow matmuls into [128,2048] PSUM slabs from resident
                # W2.  Consumers alternate ACT (fused exp+rowsum in place)
                # and DVE (Schraudolph fast-exp) over the early slabs so
                # adjacent slabs rarely share a consumer: a single consumer
                # (~2.1-2.3us PSUM hold) is slower than the 1.73us PE fill
                # and would stall the 2-buffer ring.  The DVE budget caps at
                # 3 slabs/tile (its SBUF-side row-sum costs another 2.2us
                # per slab); the short tail slab stays on ACT (1.45us exp,
                # under the fill time).  t=0 keeps the DVE lighter while it
                # finishes phase-1 work.
                v0 = 0
                for vg, vgw in enumerate(VGROUPS):
                    ps = psp.tile([128, VG], F32, tag="ps")
                    # kc-outer so the stationary hT slice is reused across
                    # the 4 column chunks of the slab.
                    for kc in range(NKP):
                        for lo, w in _chunks(vgw):
                            nc.tensor.matmul(
                                ps[:, lo : lo + w],
                                lhsT=hT[:, 2 * kc : 2 * kc + 2, ts(t, 128)],
                                rhs=w2tiles[vg][:, 2 * kc : 2 * kc + 2, lo : lo + w],
                                start=(kc == 0),
                                stop=(kc == NKP - 1),
                                perf_mode=DR,
                            )
                    if b2_nz:
                        nc.vector.tensor_add(
                            ps[:, :vgw], ps[:, :vgw], b2sb[:, v0 : v0 + vgw]
                        )
                    acc = sums[:, t * NVG + vg : t * NVG + vg + 1]
                    if vg in ((7,) if t == 0 else (3, 7)):
                        scr = scrp.tile([128, VG], I32, tag="scr", bufs=3)
                        nc.vector.tensor_scalar(
                            out=scr[:, :vgw],
                            in0=ps[:, :vgw],
                            scalar1=SCH_A,
                            scalar2=SCH_B,
                            op0=OP.mult,
                            op1=OP.add,
                        )
                        nc.vector.reduce_sum(
                            out=acc, in_=scr[:, :vgw].bitcast(F32),
                            axis=mybir.AxisListType.X,
                        )
                    else:
                        nc.scalar.activation(
                            ps[:, :vgw], ps[:, :vgw], AF.Exp,
                            scale=1.0 / W2SCALE,
                            accum_out=acc,
                        )
                    v0 += vgw

            # Interleaved emission: PE starts phase 2 for the first half's
            # batch tiles after only half-0's gathers; the second half's
            # phase 1, the target gathers, and the target dot overlap it.
            # (Emission order = engine queue order = scheduler priority; the
            # coalesced semaphore waits make anything emitted earlier on a
            # producer engine gate everything later on its consumers.)
            for t in range(4):
                gather_cast(t)
            # identity after the first gathers: make_identity runs on the
            # same serial GPSIMD engine and would delay them ~1.5us
            make_identity(nc, ident[:])
            # W2 preload: per slab-group tile, 4 DMAs each (one per hc-plane)
            # so a chunk lands ~4x faster than one queue could deliver it,
            # emitted slab-major so arrival order matches consumption
            # (sync-DMA queues, disjoint from the GPSIMD gather queue).
            w2r = w2.rearrange("(c p) v -> p c v", p=128)
            v0 = 0
            for vg, vgw in enumerate(VGROUPS):
                for c in range(NHC):
                    nc.sync.dma_start(
                        w2tiles[vg][:, c, :], w2r[:, c, v0 : v0 + vgw]
                    )
                v0 += vgw
            phase1a_block(0, 4)
            for t in range(4, NBT):
                gather_cast(t)
            gather_targets()
            phase2_pair(0, 1)
            phase1a_block(4, 4)
            phase1_hb_block(0, 4)
            phase2_t(2)
            phase1_hb_block(4, 4)
            phase1b_tdot()
            for t in range(3, NBT):
                phase2_t(t)

            # ---- Phase 3: logsumexp and output, entirely on the DVE.
            # ln(S) = (y + R(m)) * ln2 with y = float(bits(S))*2^-23 - 127
            # (= e + m - 1), m the mantissa in [1,2) extracted by one fused
            # and/or bit-op, and R a deg-4 fit of log2(m)-m+1 (~2e-5 abs
            # error; NLL tol is ~0.15).  Using AF.Ln instead would reload
            # the ACT table mid-exp-stream at every batch-tile boundary
            # (the scheduler hoists each tile's Ln up into phase 2). ----
            p4, p3, p2, p1, p0 = _LOG_POLY
            S = fin[:, 0 * NBT : 1 * NBT]
            y = fin[:, 1 * NBT : 2 * NBT]
            m = fin[:, 2 * NBT : 3 * NBT]
            a0 = fin[:, 3 * NBT : 4 * NBT]
            a1 = fin[:, 4 * NBT : 5 * NBT]
            res = fin[:, 5 * NBT : 6 * NBT]
            for t in range(NBT):
                nc.vector.reduce_sum(
                    out=S[:, t : t + 1],
                    in_=sums[:, ts(t, NVG)],
                    axis=mybir.AxisListType.X,
                )
            nc.vector.tensor_copy(y, S.bitcast(I32))   # float(raw bits)
            nc.vector.tensor_scalar(
                out=y, in0=y, scalar1=2.0 ** -23, scalar2=-127.0,
                op0=OP.mult, op1=OP.add,
            )
            nc.vector.tensor_scalar(
                out=m.bitcast(I32), in0=S.bitcast(I32),
                scalar1=0x007FFFFF, scalar2=0x3F800000,
                op0=OP.bitwise_and, op1=OP.bitwise_or,
            )
            # monic Horner with fused (acc+c)*m stages; p4 and ln2 fold into
            # the last two ops, minimizing the serial DVE chain (each DVE op
            # trails a ~266ns pipe drain).
            stt = nc.vector.scalar_tensor_tensor
            stt(a0, m, p3 / p4, m, op0=OP.add, op1=OP.mult)
            stt(a1, a0, p2 / p4, m, op0=OP.add, op1=OP.mult)
            stt(a0, a1, p1 / p4, m, op0=OP.add, op1=OP.mult)
            nc.vector.tensor_scalar(
                out=a1, in0=a0, scalar1=LN2 * p4, scalar2=LN2 * p0,
                op0=OP.mult, op1=OP.add,
            )
            stt(res, y, LN2, a1, op0=OP.mult, op1=OP.add)
            nc.vector.tensor_sub(res, res, tdot[:])
            nc.sync.dma_start(nll, res)

    nc.compile()
    _BUILD_CACHE[key] = nc
    return nc


def _prep_inputs(ws, cs, vectors, W1, b1, W2, b2, vector_to_support):
    ws = np.asarray(ws)
    cs = np.asarray(cs)
    vectors = np.asarray(vectors, dtype=np.float32)
    W1 = np.asarray(W1, dtype=np.float32)
    b1 = np.asarray(b1, dtype=np.float32)
    W2 = np.asarray(W2, dtype=np.float32)
    b2 = np.asarray(b2, dtype=np.float32)
    v2s = np.asarray(vector_to_support)

    b1_nz = bool(np.any(b1))
    b2_nz = bool(np.any(b2))

    w1p = np.zeros((DP, H), dtype=ml_dtypes.bfloat16)
    w1p[:D] = W1.astype(ml_dtypes.bfloat16)
    # fp8 W2, scaled so values land in the e4m3 normal range (TRN e4m3
    # matches OCP e4m3fn bit patterns for |x| <= 240).
    w2f8 = np.ascontiguousarray(
        np.clip(W2 * W2SCALE, -240.0, 240.0).astype(ml_dtypes.float8_e4m3fn)
    )
    w2tb = np.ascontiguousarray(
        np.concatenate([W2.T, b2[:, None]], axis=1).astype(np.float32)
    )
    v2s2d = np.ascontiguousarray(v2s.astype(np.int32).reshape(N_VOCAB, 1))

    shared = {
        "vectors": np.ascontiguousarray(vectors),
        "v2s": v2s2d,
        "w1": w1p,
        "w2": w2f8,
        "w2tb": w2tb,
    }
    if b1_nz:
        shared["b1c"] = np.ascontiguousarray(b1.reshape(NHC, 128, 1))
        shared["b1rep"] = np.ascontiguousarray(
            np.broadcast_to(b1, (128, H)).astype(np.float32)
        )
    if b2_nz:
        shared["b2rep"] = np.ascontiguousarray(
            np.broadcast_to(b2 * W2SCALE, (128, V)).astype(np.float32)
        )

    in_maps = []
    for c in range(NCORES):
        sl = slice(c * BL, (c + 1) * BL)
        m = dict(shared)
        m["cs_idx"] = np.ascontiguousarray(
            cs[sl].astype(np.int32).reshape(NBT, 128, 1)
        )
        m["ws_idx"] = np.ascontiguousarray(
            ws[sl].astype(np.int32).reshape(NBT, 128, 1)
        )
        in_maps.append(m)
    return in_maps, b1_nz, b2_nz


def run(inputs: dict, trace: bool = False):
    """Run the SPMD kernel. Returns (output [B] fp32, BassKernelResults)."""
    in_maps, b1_nz, b2_nz = _prep_inputs(**inputs)
    nc = _build(b1_nz, b2_nz)
    res = bass_utils.run_bass_kernel_spmd(
        nc, in_maps, core_ids=list(range(NCORES)), trace=trace
    )
    out = np.concatenate(
        [np.ascontiguousarray(r["nll"].T).reshape(-1) for r in res.results]
    ).astype(np.float32)
    return out, res


def kernel(**inputs) -> np.ndarray:
    out, _ = run(inputs, trace=False)
    return out


# revision 52
# speedup vs baseline: 1.0118x; 1.0118x over previous
"""Trainium2 Bass kernel for nn_ConditionalSoftmax (sampled-softmax NLL loss).

Computes, for each batch row b:
    v_c   = vectors[cs[b]]                      # [D]
    h     = relu(v_c @ W1 + b1)                 # [H]
    logit = h @ W2 + b2                         # [V]
    nll_b = logsumexp(logit) - logit[v2s[ws[b]]]

Sharding: data-parallel over batch across 8 NeuronCores (1024 rows/core),
weights replicated.  Per core the dominant work is the [1024,512]@[512,20000]
matmul; it runs in fp8(e4m3) DoubleRow perf mode (2 fp8 weights per PE cell,
2 MACs/cycle), with W2 pre-scaled by 16 on the host so its values sit in the
e4m3 normal range.  W2 (10 MB in fp8) is preloaded once into SBUF and stays
resident; phase 2 runs batch-tile-outer with zero DMA.  Logits accumulate in
PSUM as [128,2048] slabs (4 banks); 7 of 10 slabs per batch tile are reduced
by the ScalarEngine's fused exp+row-sum (accum_out, scale=1/16 undoes the W2
scaling, written back in place over PSUM to avoid the ACT SBUF write bubble),
and 3 are reduced on the VectorEngine with a bias-tuned Schraudolph fast exp
(int bit-trick; only ~1e-4 relative error on each slab *sum*), so the two
engines together keep pace with the PE and the [1024,20000] logit matrix
never touches HBM.  The target logit takes a separate exact path:
indirect-gather of the needed W2.T rows (fp32) and a multiply-reduce on the
VectorEngine against an fp32 recompute of h, keeping NLL error ~1e-3.

All GPSIMD indirect-gather ops are emitted before the first activation in
program order: their DGE ring-management branches split the CFG, and any
split between activations makes the conservative act-table-load pass reload
the 1.3us exp table at every boundary.
"""

import numpy as np
import ml_dtypes

import concourse.bass as bass
import concourse.mybir as mybir
import concourse.tile as tile
from concourse import bacc, bass_utils
from concourse.bass import IndirectOffsetOnAxis, ts
from concourse.masks import make_identity

# Problem shapes (hardcoded per contest contract)
N_VOCAB = 50000
V = 20000
D = 300
DP = 384          # D padded to 3*128
NDC = 3           # contraction chunks for D
H = 512
NHC = 4           # contraction chunks for H (128 each)
NKP = 2           # DoubleRow contraction pairs (256 each)
B = 8192
NCORES = 8
BL = B // NCORES  # 1024 rows per core
NBT = BL // 128   # 8 batch tiles of 128 rows
W2SCALE = 16.0    # host-side scale on fp8 W2; undone by the Exp pre-scale

# Phase-2 vocab grouping: PSUM slabs of 2048 fp32 (4 banks), matmul chunks
# of <=512 so no matmul output crosses a PSUM bank. 20000 = 9*2048 + 1568.
VG = 2048
VGROUPS = [VG] * 9 + [20000 - 9 * VG]   # last = 1568
NVG = len(VGROUPS)
NW2DMA = 20       # W2 preload split for DMA-queue parallelism

F32 = mybir.dt.float32
BF16 = mybir.dt.bfloat16
FP8 = mybir.dt.float8e4
I32 = mybir.dt.int32
AF = mybir.ActivationFunctionType
OP = mybir.AluOpType
DR = mybir.MatmulPerfMode.DoubleRow


# Degree-4 least-squares fit of R(m) = log2(m) - m + 1 on [1,2], used by the
# DVE log path in phase 3 (computing ln on the ScalarEngine would force a
# ~1.3us act-table switch away from the exp table at every tile boundary).
_m_grid = np.linspace(1.0, 2.0, 100001)
_LOG_POLY = np.polyfit(_m_grid, np.log2(_m_grid) - _m_grid + 1.0, 4).tolist()
LN2 = float(np.log(2.0))


def _schraudolph_consts():
    """Constants for exp(x/16) ~= bitcast_f32(int(x*A + B)) on PSUM values
    x = 16*logit, used by the DVE fast-exp that reduces one slab per batch
    tile.  B's offset is tuned (fp64, through an fp32-faithful pipeline) so
    the *mean* multiplicative error over the logit distribution is ~0,
    making slab-sum errors ~1e-4 instead of the pointwise ~3%."""
    ln2 = float(np.log(2.0))
    l = np.linspace(-3.5, 3.5, 400001, dtype=np.float64)
    w = np.exp(-((l / 0.72) ** 2) / 2.0) * np.exp(l)
    A = (2.0 ** 23) / ln2
    C = 0.0
    for _ in range(10):
        v = np.float32(l * A + (127.0 * 2 ** 23 - C)).astype(np.float64)
        y = np.round(v).astype(np.int64).astype(np.uint32).view(np.float32)
        r = float(np.average(y.astype(np.float64) / np.exp(l), weights=w))
        if abs(r - 1.0) < 1e-9:
            break
        C += np.log2(r) * 2 ** 23
    return float(A / W2SCALE), float(127.0 * 2 ** 23 - C)


SCH_A, SCH_B = _schraudolph_consts()

_BUILD_CACHE = {}


def _chunks(width):
    """512-wide matmul chunks covering [0, width)."""
    out = []
    lo = 0
    while lo < width:
        w = min(512, width - lo)
        out.append((lo, w))
        lo += w
    return out


def _build(b1_nz: bool, b2_nz: bool):
    key = (b1_nz, b2_nz)
    if key in _BUILD_CACHE:
        return _BUILD_CACHE[key]

    nc = bacc.Bacc(
        "TRN2",
        target_bir_lowering=False,
        debug=False,
        num_devices=NCORES,
        num_swdge_queues=4,
    )

    cs_idx = nc.dram_tensor("cs_idx", [NBT, 128, 1], I32, kind="ExternalInput").ap()
    ws_idx = nc.dram_tensor("ws_idx", [NBT, 128, 1], I32, kind="ExternalInput").ap()
    vectors = nc.dram_tensor("vectors", [N_VOCAB, D], F32, kind="ExternalInput").ap()
    v2s = nc.dram_tensor("v2s", [N_VOCAB, 1], I32, kind="ExternalInput").ap()
    w1 = nc.dram_tensor("w1", [DP, H], BF16, kind="ExternalInput").ap()
    w2 = nc.dram_tensor("w2", [H, V], FP8, kind="ExternalInput").ap()
    w2tb = nc.dram_tensor("w2tb", [V, H + 1], F32, kind="ExternalInput").ap()
    if b1_nz:
        b1c = nc.dram_tensor("b1c", [NHC, 128, 1], F32, kind="ExternalInput").ap()
        b1rep = nc.dram_tensor("b1rep", [128, H], F32, kind="ExternalInput").ap()
    if b2_nz:
        b2rep = nc.dram_tensor("b2rep", [128, V], F32, kind="ExternalInput").ap()
    # partition-major [128, NBT]: one contiguous output DMA; host transposes
    nll = nc.dram_tensor("nll", [128, NBT], F32, kind="ExternalOutput").ap()

    with tile.TileContext(nc) as tc:
        with (
            tc.tile_pool(name="consts", bufs=1) as consts,
            tc.tile_pool(name="idx", bufs=8) as idxp,
            tc.tile_pool(name="vc", bufs=8) as vcp,
            tc.tile_pool(name="gw", bufs=4) as gwp,
            tc.tile_pool(name="scr", bufs=2) as scrp,
            # Single PSUM pool/tag: 2 bufs x [128,2048] fp32 = all 8 banks.
            # Phase-1 tiles draw smaller shapes from the same tag.
            tc.tile_pool(name="ps", bufs=2, space="PSUM") as psp,
        ):
            ident = consts.tile([128, 128], BF16)
            w1sb = consts.tile([128, NDC, H], BF16)
            if b1_nz:
                b1sb = consts.tile([128, NHC], F32)
                for hc in range(NHC):
                    nc.sync.dma_start(b1sb[:, hc : hc + 1], b1c[hc])
                b1rep_sb = consts.tile([128, H], F32)
                nc.sync.dma_start(b1rep_sb[:], b1rep[:])

            # Long-lived activations / resident weights
            w2sb = consts.tile([128, NHC, V], FP8)     # all of W2, resident
            vcT = consts.tile([128, NDC, BL], BF16)    # v_c^T, d-major
            hT = consts.tile([128, NHC, BL], FP8)      # h^T, h-major (PE fp8 input)
            hb = consts.tile([128, NBT, H], F32)       # h, batch-major (target dot)
            sums = consts.tile([128, NBT * NVG], F32)  # per-(b,vg) exp partial sums
            tdot = consts.tile([128, NBT], F32)        # target logits
            fin = consts.tile([128, 7 * NBT], F32)     # DVE-log scratch columns
            if b2_nz:
                b2sb = consts.tile([128, V], F32)
                nc.sync.dma_start(b2sb[:], b2rep[:])

            # ---- Phase 1 front helpers.  All GPSIMD indirect gathers must be
            # emitted before the first activation (see module docstring), and
            # the half-0 compute must be emitted before the half-1 gathers:
            # the tile scheduler coalesces semaphore waits over everything
            # emitted earlier on the producer engine, so any gather emitted
            # before the first transpose would gate the whole PE pipeline. ----
            cidxs, vcs, vcbs, gs = [], [], [], [None] * NBT
            for t in range(NBT):
                cidx = idxp.tile([128, 1], I32, tag="cidx")
                nc.sync.dma_start(cidx[:], cs_idx[t])
                cidxs.append(cidx)
            # w1 load after the cidx DMAs: anything queued ahead of them
            # delays the first gather (waits are coalesced per queue).
            w1r = w1.rearrange("(c p) h -> p c h", p=128)
            for c in range(NDC):
                nc.sync.dma_start(w1sb[:, c, :], w1r[:, c, :])

            def gather_cast(t):
                vc = vcp.tile([128, D], F32, tag="vc")
                nc.gpsimd.indirect_dma_start(
                    out=vc[:],
                    out_offset=None,
                    in_=vectors[:],
                    in_offset=IndirectOffsetOnAxis(ap=cidxs[t][:, :1], axis=0),
                )
                vcb = vcp.tile([128, DP], BF16, tag="vcb")
                nc.vector.memset(vcb[:, D:DP], 0.0)
                nc.vector.tensor_copy(vcb[:, :D], vc[:])
                vcs.append(vc)
                vcbs.append(vcb)

            def gather_targets():
                for t in range(NBT):
                    widx = idxp.tile([128, 1], I32, tag="widx")
                    nc.sync.dma_start(widx[:], ws_idx[t])
                    sidx = idxp.tile([128, 1], I32, tag="sidx")
                    nc.gpsimd.indirect_dma_start(
                        out=sidx[:],
                        out_offset=None,
                        in_=v2s[:],
                        in_offset=IndirectOffsetOnAxis(ap=widx[:, :1], axis=0),
                    )
                    g = gwp.tile([128, H + 1], F32, tag="g", bufs=8)
                    nc.gpsimd.indirect_dma_start(
                        out=g[:],
                        out_offset=None,
                        in_=w2tb[:],
                        in_offset=IndirectOffsetOnAxis(ap=sidx[:, :1], axis=0),
                    )
                    gs[t] = g

            def phase1a_block(tb, n_t):
                # transposes + first layer for batch tiles tb..tb+n_t-1; all
                # transposes land in ONE psum tile (c-major) so the PSUM ring
                # doesn't ping-pong PE<->DVE per 128-col tile, then one wide
                # copy moves them to vcT.  h^T in fp8 slabs [128h x 128*n_t b];
                # relu+bias+cast on the DVE.  Called with n_t=1 for t=0 so
                # phase 2 starts after a single gather.
                w = 128 * n_t
                lo = 128 * tb
                pt3 = psp.tile([128, NDC, w], BF16, tag="ps")
                for c in range(NDC):
                    for j in range(n_t):
                        nc.tensor.transpose(
                            pt3[:, c, ts(j, 128)],
                            vcbs[tb + j][:, ts(c, 128)],
                            ident[:],
                        )
                nc.vector.tensor_copy(vcT[:, :, lo : lo + w], pt3[:])
                for hc in range(NHC):
                    ph = psp.tile([128, w], F32, tag="ps")
                    for c in range(NDC):
                        nc.tensor.matmul(
                            ph[:],
                            lhsT=w1sb[:, c, ts(hc, 128)],
                            rhs=vcT[:, c, lo : lo + w],
                            start=(c == 0),
                            stop=(c == NDC - 1),
                        )
                    if b1_nz:
                        nc.vector.tensor_scalar(
                            out=hT[:, hc, lo : lo + w],
                            in0=ph[:],
                            scalar1=b1sb[:, hc : hc + 1],
                            scalar2=0.0,
                            op0=OP.add,
                            op1=OP.max,
                        )
                    else:
                        nc.vector.tensor_scalar_max(
                            hT[:, hc, lo : lo + w], ph[:], 0.0
                        )

            def phase1_hb_block(tb, n_t):
                # batch-major h (fp32) for the target-logit dot; emitted
                # after later phase2_t calls so it doesn't delay the
                # first exp slabs (hb is only needed by the target dot).
                for t in range(tb, tb + n_t):
                    phb = psp.tile([128, H], F32, tag="ps")
                    for c in range(NDC):
                        nc.tensor.matmul(
                            phb[:],
                            lhsT=vcT[:, c, ts(t, 128)],
                            rhs=w1sb[:, c, :],
                            start=(c == 0),
                            stop=(c == NDC - 1),
                        )
                    if b1_nz:
                        nc.vector.tensor_add(phb[:], phb[:], b1rep_sb[:])
                    nc.vector.tensor_scalar_max(hb[:, t, :], phb[:], 0.0)

            def phase1b_tdot():
                # target-logit dot on the DVE; emitted after phase2_t(1) so
                # these ops don't sit in the DVE queue ahead of the copies /
                # relus that gate the PE pipeline during phase 1.
                for t in range(NBT):
                    # (tensor_tensor_reduce is broken on this HW; use 3 ops)
                    gscr = gwp.tile([128, H], F32, tag="gscr")
                    nc.vector.tensor_mul(gscr[:], hb[:, t, :], gs[t][:, :H])
                    gacc = gwp.tile([128, 1], F32, tag="gacc")
                    nc.vector.reduce_sum(
                        out=gacc[:], in_=gscr[:], axis=mybir.AxisListType.X
                    )
                    nc.vector.tensor_add(
                        tdot[:, t : t + 1], gacc[:], gs[t][:, H : H + 1]
                    )

            def phase2_slab(t, vg, vgw, v0, dve):
                ps = psp.tile([128, VG], F32, tag="ps")
                # kc-outer so the stationary hT slice is reused across
                # the 4 column chunks of the slab.
                for kc in range(NKP):
                    for lo, w in _chunks(vgw):
                        nc.tensor.matmul(
                            ps[:, lo : lo + w],
                            lhsT=hT[:, 2 * kc : 2 * kc + 2, ts(t, 128)],
                            rhs=w2sb[:, 2 * kc : 2 * kc + 2, v0 + lo : v0 + lo + w],
                            start=(kc == 0),
                            stop=(kc == NKP - 1),
                            perf_mode=DR,
                        )
                if b2_nz:
                    nc.vector.tensor_add(
                        ps[:, :vgw], ps[:, :vgw], b2sb[:, v0 : v0 + vgw]
                    )
                acc = sums[:, t * NVG + vg : t * NVG + vg + 1]
                if dve:
                    scr = scrp.tile([128, VG], I32, tag="scr", bufs=3)
                    nc.vector.tensor_scalar(
                        out=scr[:, :vgw],
                        in0=ps[:, :vgw],
                        scalar1=SCH_A,
                        scalar2=SCH_B,
                        op0=OP.mult,
                        op1=OP.add,
                    )
                    nc.vector.reduce_sum(
                        out=acc, in_=scr[:, :vgw].bitcast(F32),
                        axis=mybir.AxisListType.X,
                    )
                else:
                    nc.scalar.activation(
                        ps[:, :vgw], ps[:, :vgw], AF.Exp,
                        scale=1.0 / W2SCALE,
                        accum_out=acc,
                    )

            def phase2_pair(ta, tb):
                # first two batch tiles interleaved vg-outer: one tile alone
                # consumes the 10MB resident W2 (~19us) faster than the
                # preload DMAs deliver it (~28us), so a t-outer start would
                # leave the PE waiting on the load.  Two tiles per vocab
                # chunk halve the consumption rate to below DMA supply.
                v0 = 0
                for vg, vgw in enumerate(VGROUPS):
                    phase2_slab(ta, vg, vgw, v0, vg in (3, 7))
                    phase2_slab(tb, vg, vgw, v0, vg in (1, 5, 9))
                    v0 += vgw

            def phase2_t(t):
                # DoubleRow matmuls into [128,2048] PSUM slabs from resident
                # W2.  Consumers alternate ACT (fused exp+rowsum in place)
                # and DVE (Schraudolph fast-exp) over the early slabs so
                # adjacent slabs rarely share a consumer: a single consumer
                # (~2.1-2.3us PSUM hold) is slower than the 1.73us PE fill
                # and would stall the 2-buffer ring.  The DVE budget caps at
                # 3 slabs/tile (its SBUF-side row-sum costs another 2.2us
                # per slab); the short tail slab stays on ACT (1.45us exp,
                # under the fill time).  t=0 keeps the DVE lighter while it
                # finishes phase-1 work.
                v0 = 0
                for vg, vgw in enumerate(VGROUPS):
                    ps = psp.tile([128, VG], F32, tag="ps")
                    # kc-outer so the stationary hT slice is reused across
                    # the 4 column chunks of the slab.
                    for kc in range(NKP):
                        for lo, w in _chunks(vgw):
                            nc.tensor.matmul(
                                ps[:, lo : lo + w],
                                lhsT=hT[:, 2 * kc : 2 * kc + 2, ts(t, 128)],
                                rhs=w2sb[:, 2 * kc : 2 * kc + 2, v0 + lo : v0 + lo + w],
                                start=(kc == 0),
                                stop=(kc == NKP - 1),
                                perf_mode=DR,
                            )
                    if b2_nz:
                        nc.vector.tensor_add(
                            ps[:, :vgw], ps[:, :vgw], b2sb[:, v0 : v0 + vgw]
                        )
                    acc = sums[:, t * NVG + vg : t * NVG + vg + 1]
                    if vg in ((7,) if t == 0 else (3, 7)):
                        scr = scrp.tile([128, VG], I32, tag="scr", bufs=3)
                        nc.vector.tensor_scalar(
                            out=scr[:, :vgw],
                            in0=ps[:, :vgw],
                            scalar1=SCH_A,
                            scalar2=SCH_B,
                            op0=OP.mult,
                            op1=OP.add,
                        )
                        nc.vector.reduce_sum(
                            out=acc, in_=scr[:, :vgw].bitcast(F32),
                            axis=mybir.AxisListType.X,
                        )
                    else:
                        nc.scalar.activation(
                            ps[:, :vgw], ps[:, :vgw], AF.Exp,
                            scale=1.0 / W2SCALE,
                            accum_out=acc,
                        )
                    v0 += vgw

            # Interleaved emission: PE starts phase 2 for the first half's
            # batch tiles after only half-0's gathers; the second half's
            # phase 1, the target gathers, and the target dot overlap it.
            # (Emission order = engine queue order = scheduler priority; the
            # coalesced semaphore waits make anything emitted earlier on a
            # producer engine gate everything later on its consumers.)
            for t in range(4):
                gather_cast(t)
            # identity after the first gathers: make_identity runs on the
            # same serial GPSIMD engine and would delay them ~1.5us
            make_identity(nc, ident[:])
            # W2 preload, split into v-chunks for DMA-queue parallelism
            # (sync-DMA queues, disjoint from the GPSIMD gather queue).
            w2r = w2.rearrange("(c p) v -> p c v", p=128)
            wv = V // NW2DMA
            for i in range(NW2DMA):
                nc.sync.dma_start(w2sb[:, :, ts(i, wv)], w2r[:, :, ts(i, wv)])
            phase1a_block(0, 4)
            for t in range(4, NBT):
                gather_cast(t)
            gather_targets()
            phase2_pair(0, 1)
            phase1a_block(4, 4)
            phase1_hb_block(0, 4)
            phase2_t(2)
            phase1_hb_block(4, 4)
            phase1b_tdot()
            for t in range(3, NBT):
                phase2_t(t)

            # ---- Phase 3: logsumexp and output, entirely on the DVE.
            # ln(S) = (y + R(m)) * ln2 with y = float(bits(S))*2^-23 - 127
            # (= e + m - 1), m the mantissa in [1,2) extracted by one fused
            # and/or bit-op, and R a deg-4 fit of log2(m)-m+1 (~2e-5 abs
            # error; NLL tol is ~0.15).  Using AF.Ln instead would reload
            # the ACT table mid-exp-stream at every batch-tile boundary
            # (the scheduler hoists each tile's Ln up into phase 2). ----
            p4, p3, p2, p1, p0 = _LOG_POLY
            S = fin[:, 0 * NBT : 1 * NBT]
            y = fin[:, 1 * NBT : 2 * NBT]
            m = fin[:, 2 * NBT : 3 * NBT]
            a0 = fin[:, 3 * NBT : 4 * NBT]
            a1 = fin[:, 4 * NBT : 5 * NBT]
            res = fin[:, 5 * NBT : 6 * NBT]
            for t in range(NBT):
                nc.vector.reduce_sum(
                    out=S[:, t : t + 1],
                    in_=sums[:, ts(t, NVG)],
                    axis=mybir.AxisListType.X,
                )
            nc.vector.tensor_copy(y, S.bitcast(I32))   # float(raw bits)
            nc.vector.tensor_scalar(
                out=y, in0=y, scalar1=2.0 ** -23, scalar2=-127.0,
                op0=OP.mult, op1=OP.add,
            )
            nc.vector.tensor_scalar(
                out=m.bitcast(I32), in0=S.bitcast(I32),
                scalar1=0x007FFFFF, scalar2=0x3F800000,
                op0=OP.bitwise_and, op1=OP.bitwise_or,
            )
            # monic Horner with fused (acc+c)*m stages; p4 and ln2 fold into
            # the last two ops, minimizing the serial DVE chain (each DVE op
            # trails a ~266ns pipe drain).
            stt = nc.vector.scalar_tensor_tensor
            stt(a0, m, p3 / p4, m, op0=OP.add, op1=OP.mult)
            stt(a1, a0, p2 / p4, m, op0=OP.add, op1=OP.mult)
            stt(a0, a1, p1 / p4, m, op0=OP.add, op1=OP.mult)
            nc.vector.tensor_scalar(
                out=a1, in0=a0, scalar1=LN2 * p4, scalar2=LN2 * p0,
                op0=OP.mult, op1=OP.add,
            )
            stt(res, y, LN2, a1, op0=OP.mult, op1=OP.add)
            nc.vector.tensor_sub(res, res, tdot[:])
            nc.sync.dma_start(nll, res)

    nc.compile()
    _BUILD_CACHE[key] = nc
    return nc


def _prep_inputs(ws, cs, vectors, W1, b1, W2, b2, vector_to_support):
    ws = np.asarray(ws)
    cs = np.asarray(cs)
    vectors = np.asarray(vectors, dtype=np.float32)
    W1 = np.asarray(W1, dtype=np.float32)
    b1 = np.asarray(b1, dtype=np.float32)
    W2 = np.asarray(W2, dtype=np.float32)
    b2 = np.asarray(b2, dtype=np.float32)
    v2s = np.asarray(vector_to_support)

    b1_nz = bool(np.any(b1))
    b2_nz = bool(np.any(b2))

    w1p = np.zeros((DP, H), dtype=ml_dtypes.bfloat16)
    w1p[:D] = W1.astype(ml_dtypes.bfloat16)
    # fp8 W2, scaled so values land in the e4m3 normal range (TRN e4m3
    # matches OCP e4m3fn bit patterns for |x| <= 240).
    w2f8 = np.ascontiguousarray(
        np.clip(W2 * W2SCALE, -240.0, 240.0).astype(ml_dtypes.float8_e4m3fn)
    )
    w2tb = np.ascontiguousarray(
        np.concatenate([W2.T, b2[:, None]], axis=1).astype(np.float32)
    )
    v2s2d = np.ascontiguousarray(v2s.astype(np.int32).reshape(N_VOCAB, 1))

    shared = {
        "vectors": np.ascontiguousarray(vectors),
        "v2s": v2s2d,
        "w1": w1p,
        "w2": w2f8,
        "w2tb": w2tb,
    }
    if b1_nz:
        shared["b1c"] = np.ascontiguousarray(b1.reshape(NHC, 128, 1))
        shared["b1rep"] = np.ascontiguousarray(
            np.broadcast_to(b1, (128, H)).astype(np.float32)
        )
    if b2_nz:
        shared["b2rep"] = np.ascontiguousarray(
            np.broadcast_to(b2 * W2SCALE, (128, V)).astype(np.float32)
        )

    in_maps = []
    for c in range(NCORES):
        sl = slice(c * BL, (c + 1) * BL)
        m = dict(shared)
        m["cs_idx"] = np.ascontiguousarray(
            cs[sl].astype(np.int32).reshape(NBT, 128, 1)
        )
        m["ws_idx"] = np.ascontiguousarray(
            ws[sl].astype(np.int32).reshape(NBT, 128, 1)
        )
        in_maps.append(m)
    return in_maps, b1_nz, b2_nz


def run(inputs: dict, trace: bool = False):
    """Run the SPMD kernel. Returns (output [B] fp32, BassKernelResults)."""
    in_maps, b1_nz, b2_nz = _prep_inputs(**inputs)
    nc = _build(b1_nz, b2_nz)
    res = bass_utils.run_bass_kernel_spmd(
        nc, in_maps, core_ids=list(range(NCORES)), trace=trace
    )
    out = np.concatenate(
        [np.ascontiguousarray(r["nll"].T).reshape(-1) for r in res.results]
    ).astype(np.float32)
    return out, res


def kernel(**inputs) -> np.ndarray:
    out, _ = run(inputs, trace=False)
    return out
